# revision 9
# baseline (speedup 1.0000x reference)
"""DeepseekMoE block-quantized MoE kernel for 8 Trainium2 NeuronCores.

Strategy (expert-parallel with host-side dispatch):
  - The routing table (selected_experts) is known on the host before launch,
    so the all-to-all "dispatch" is done on the host: for each expert e we
    gather the unique tokens routed to it (dedup across the top-k slots),
    transpose to [H, n_e], and pad to a common capacity C.
  - Experts are sharded 2-per-core across the 8 cores.  Each core runs a
    dense 3-matmul MLP (gate/up -> silu*up -> down) for its 2 experts in
    x^T / act^T layout so no on-device transposes are needed.
  - Block-dequantization (w * repeat(s, 128)) is folded into the host-side
    weight preparation, which also rounds weights and x to bf16.
  - bf16 matmuls stream 1 column/cycle and enable Fast Weight Load
    (LDWEIGHTS ~53ns vs ~187ns for fp32r), so the stationary reload fully
    hides behind the moving-operand stream.  Accuracy budget: ~5e-3 L2.
  - Weights live in DRAM pre-swizzled into the exact SBUF slab layout so
    each slab load is one contiguous-per-partition DMA (4KB descriptors).
  - The host scatters the per-expert outputs back to [T, K, H].
"""

import math

import numpy as np

T = 4096
TOPK = 6
E = 16
H = 2048
I = 1408
BS = 128           # quant block size
HT = H // 128      # 16 h-tiles
IT = I // 128      # 11 i-tiles
NCORES = 8
# Single-pass SBUF budget bound: (HT + IT) * 2 * W bytes of x+act per
# partition plus ~50KB of weight/output staging must fit in ~208KB.
MAX_W = 2880

_BUILT = {}
LAST_RESULTS = None  # stashed BassKernelResults for external harnesses


def _chunk_plan(width):
    """Split `width` columns into PSUM-bank-sized chunks (<=512)."""
    if width <= 512:
        return [(0, width)]
    n = -(-width // 512)
    # 8-aligned chunk widths
    base = (width // n) // 8 * 8
    rem8 = (width - n * base) // 8
    out, off = [], 0
    for j in range(n):
        w = base + (8 if j < rem8 else 0)
        if j == n - 1:
            w = width - off
        out.append((off, w))
        off += w
    return out


def _build(jobs, CT):
    """Build the SPMD Bass program.  `jobs` is a tuple of
    (slot, col_offset, width): each job runs one expert slot's MLP over a
    window of `width` token columns; CT is the column capacity of xt/yt."""
    import concourse.bacc as bacc
    import concourse.mybir as mybir
    from concourse.bass import ts
    from concourse.tile import TileContext

    f32 = mybir.dt.float32
    bf16 = mybir.dt.bfloat16
    AF = mybir.ActivationFunctionType
    import os as _os

    act_fn = (
        AF.Sigmoid if _os.environ.get("KERNEL_SIM_SIGMOID") else AF.Silu
    )  # CoreSim lacks Silu; HW path always uses Silu

    nc = bacc.Bacc()
    xt = nc.declare_dram_parameter("xt", [2, HT, 128, CT], bf16, isOutput=False)
    # slab layouts: w0t[s, i, p, h*128+j] = W0deq[i*128+j, h*128+p]
    #               w2t[s, h, p, i*128+j] = W2deq[h*128+j, i*128+p]
    w0t = nc.declare_dram_parameter("w0t", [2, IT, 128, H], bf16, isOutput=False)
    w1t = nc.declare_dram_parameter("w1t", [2, IT, 128, H], bf16, isOutput=False)
    w2t = nc.declare_dram_parameter("w2t", [2, HT, 128, I], bf16, isOutput=False)
    yt = nc.declare_dram_parameter("yt", [2, HT, 128, CT], f32, isOutput=True)

    with TileContext(nc) as tc:
        with (
            tc.tile_pool(name="xp", bufs=1) as xp,
            tc.tile_pool(name="ap", bufs=1) as apool,
            tc.tile_pool(name="wp", bufs=2) as wp,
            tc.tile_pool(name="yp", bufs=2) as yp,
            tc.tile_pool(name="ps", bufs=2, space="PSUM") as ps,
        ):
            def load_w01_slab(which, src, i, eng=None, graded=False):
                slab = wp.tile([128, H], bf16, tag=which, name=None)
                eng = eng or nc.sync
                if graded:
                    # Prefix pieces so the first LDWEIGHTS only waits on
                    # the first 128 columns, not the whole slab.
                    for off, ln in ((0, 128), (128, 384), (512, 512), (1024, 1024)):
                        eng.dma_start(
                            out=slab[:, off : off + ln],
                            in_=src[s, i, :, off : off + ln],
                        )
                else:
                    eng.dma_start(out=slab, in_=src[s, i])
                return slab

            # HAM warm-up: the PE clock-gate defaults to 4/8 (1.2 GHz) and
            # only releases after ~3.4us of sustained PE activity.  A short
            # burst of dependency-free matmuls on a scratch tile warms the
            # array while the DMA preamble runs, so the first real matmuls
            # execute at 2.4 GHz.
            warm_sb = xp.tile([128, 512], bf16, tag="warm_sb")
            nc.vector.memset(warm_sb, 0)
            for _ in range(8):
                wps = ps.tile([128, 512], f32, tag="warm")
                nc.tensor.matmul(
                    wps, warm_sb[:, 0:128], warm_sb, start=True, stop=True
                )

            for jn, (s, co, W) in enumerate(jobs):
                    chunks = _chunk_plan(W)
                    # One merged tile for x (all 16 h-tiles) and one for the
                    # activations (all 11 i-tiles): 1 DMA instruction per
                    # chunk instead of 16, and far fewer teardown events.
                    xs_all = xp.tile([128, HT * W], bf16, tag="x", name=f"x_{jn}")
                    xs3 = xs_all.rearrange("p (h w) -> p h w", h=HT)

                    def xsl(h, c0, cw):
                        return xs_all[:, h * W + c0 : h * W + c0 + cw]

                    # Bandwidth-priority emission across the three DMA
                    # issue paths (sync HWDGE / scalar HWDGE / gpsimd
                    # SWDGE), each with its own hardware queue:
                    #   job 0:  sync:   x c0 h0-3, x c0 h4-7
                    #           scalar: w0[0:128], w0[rest], x c0 h8-15, w1
                    #           gpsimd: x c1, x c2
                    #   job >0: sync:   w0, w1 (ahead of phase-A slabs)
                    #           gpsimd: x c0, x c1, x c2
                    # The bulk x transfers ride the otherwise idle gpsimd
                    # queue so they never wedge the just-in-time w2-slab
                    # pipeline on sync during the previous job's phase B.
                    def xsrc(c0, cw):
                        return xt[s, :, :, co + c0 : co + c0 + cw].rearrange(
                            "h p w -> p h w"
                        )

                    if jn == 0:
                        w0s_first = wp.tile([128, H], bf16, tag="w0")
                        nc.scalar.dma_start(
                            out=w0s_first[:, 0:128], in_=w0t[s, 0, :, 0:128]
                        )
                        c0, cw = chunks[0]
                        nc.sync.dma_start(
                            out=xs3[:, 0:4, c0 : c0 + cw], in_=xsrc(c0, cw)[:, 0:4]
                        )
                        nc.scalar.dma_start(
                            out=w0s_first[:, 128:H], in_=w0t[s, 0, :, 128:H]
                        )
                        nc.sync.dma_start(
                            out=xs3[:, 4:8, c0 : c0 + cw], in_=xsrc(c0, cw)[:, 4:8]
                        )
                        nc.scalar.dma_start(
                            out=xs3[:, 8:16, c0 : c0 + cw], in_=xsrc(c0, cw)[:, 8:16]
                        )
                        w1s_first = load_w01_slab("w1", w1t, 0, eng=nc.scalar)
                        for c0, cw in chunks[1:]:
                            nc.gpsimd.dma_start(
                                out=xs3[:, :, c0 : c0 + cw], in_=xsrc(c0, cw)
                            )
                    else:
                        w0s_first = load_w01_slab("w0", w0t, 0)
                        w1s_first = load_w01_slab("w1", w1t, 0)
                        for c0, cw in chunks:
                            nc.gpsimd.dma_start(
                                out=xs3[:, :, c0 : c0 + cw], in_=xsrc(c0, cw)
                            )
                    acts_all = apool.tile(
                        [128, IT * W], bf16, tag="a", name=f"a_{jn}"
                    )

                    def asl(i, c0, cw):
                        return acts_all[:, i * W + c0 : i * W + c0 + cw]

                    # Phase A: gate/up projections + silu*up, per i-tile.
                    for i in range(IT):
                        if i == 0:
                            w0s, w1s = w0s_first, w1s_first
                        else:
                            w0s = load_w01_slab("w0", w0t, i)
                            w1s = load_w01_slab("w1", w1t, i)
                        for c0, cw in chunks:
                            g = ps.tile([128, 512], f32, tag="g")
                            for h in range(HT):
                                nc.tensor.matmul(
                                    g[:, :cw],
                                    w0s[:, ts(h, 128)],
                                    xsl(h, c0, cw),
                                    start=(h == 0),
                                    stop=(h == HT - 1),
                                )
                            u = ps.tile([128, 512], f32, tag="u")
                            for h in range(HT):
                                nc.tensor.matmul(
                                    u[:, :cw],
                                    w1s[:, ts(h, 128)],
                                    xsl(h, c0, cw),
                                    start=(h == 0),
                                    stop=(h == HT - 1),
                                )
                            a_sl = asl(i, c0, cw)
                            nc.scalar.activation(a_sl, g[:, :cw], act_fn)
                            nc.vector.tensor_mul(a_sl, a_sl, u[:, :cw])

                    # Phase B: down projection, per h-tile.  w2 slabs stay
                    # alone on the sync queue (full prefetch depth); y
                    # stores issue from the vector queue right behind their
                    # PSUM->SBUF copies.
                    for h in range(HT):
                        w2s = wp.tile([128, I], bf16, tag="w2", bufs=6)
                        nc.sync.dma_start(out=w2s, in_=w2t[s, h])
                        for c0, cw in chunks:
                            o = ps.tile([128, 512], f32, tag="o")
                            for i in range(IT):
                                nc.tensor.matmul(
                                    o[:, :cw],
                                    w2s[:, ts(i, 128)],
                                    asl(i, c0, cw),
                                    start=(i == 0),
                                    stop=(i == IT - 1),
                                )
                            yc = yp.tile([128, 512], f32, tag="y")
                            nc.vector.tensor_copy(yc[:, :cw], o[:, :cw])
                            nc.scalar.dma_start(
                                out=yt[s, h, :, co + c0 : co + c0 + cw],
                                in_=yc[:, :cw],
                            )
    nc.finalize()
    return nc


def _get_built(jobs, CT):
    key = (tuple(jobs), CT)
    if key not in _BUILT:
        _BUILT[key] = _build(tuple(jobs), CT)
    return _BUILT[key]


def _dequant(w, s):
    """w: [E, O, Iin], s: [E, O, Iin//128] -> dequantized [E, O, Iin]."""
    e, o, iin = w.shape
    return (w.reshape(e, o, iin // BS, BS) * s[..., None]).reshape(e, o, iin)


def kernel(**inputs):
    global LAST_RESULTS
    import ml_dtypes

    bf16 = ml_dtypes.bfloat16

    x = np.ascontiguousarray(np.asarray(inputs["x"], dtype=np.float32))
    sel = np.asarray(inputs["selected_experts"])
    w0 = np.asarray(inputs["w0"], dtype=np.float32)
    s0 = np.asarray(inputs["s0"], dtype=np.float32)
    w1 = np.asarray(inputs["w1"], dtype=np.float32)
    s1 = np.asarray(inputs["s1"], dtype=np.float32)
    w2 = np.asarray(inputs["w2"], dtype=np.float32)
    s2 = np.asarray(inputs["s2"], dtype=np.float32)

    t, k = sel.shape
    assert (t, k) == (T, TOPK) and x.shape == (T, H)

    # ---- host-side dispatch: unique tokens per expert ----
    pos = np.full((E, T), -1, dtype=np.int32)
    cols = []
    for e in range(E):
        toks = np.nonzero((sel == e).any(axis=1))[0]
        cols.append(toks)
        pos[e, toks] = np.arange(len(toks), dtype=np.int32)
    counts = np.array([len(c) for c in cols])

    # Assign experts to (core, slot): slot 0 holds the 8 largest experts,
    # slot 1 the 8 smallest, so each slot's padded width is only the max of
    # its own rank group.  expert_of[s][c] = expert on core c, slot s.
    order = np.argsort(-counts, kind="stable")
    expert_of = [list(order[:NCORES]), list(order[NCORES:])]

    def align8(v):
        return max(256, -(-v // 8) * 8)

    slot_w = [align8(int(counts[expert_of[s]].max())) for s in range(2)]

    if max(slot_w) <= MAX_W:
        jobs = tuple((s, 0, slot_w[s]) for s in range(2))
        CT = max(slot_w)
    else:
        # fallback: uniform width, multiple column windows per slot
        cmax = int(counts.max())
        passes = max(1, math.ceil(cmax / MAX_W))
        W = align8(math.ceil(cmax / passes))
        CT = W * passes
        jobs = tuple((s, cp * W, W) for s in range(2) for cp in range(passes))

    # ---- dequantize + swizzle weights into SBUF slab layout (host) ----
    # w0/w1 slabs: [E, IT, 128(p=h-inner), HT*128(j of i-tile ... )]
    #   slab[e, it, p, ht*128+jj] = W0deq[e, it*128+jj, ht*128+p]
    W0d = _dequant(w0, s0)  # [E, I, H]
    W1d = _dequant(w1, s1)  # [E, I, H]
    W2d = _dequant(w2, s2)  # [E, H, I]
    w0slab = np.ascontiguousarray(
        W0d.reshape(E, IT, 128, HT, 128).transpose(0, 1, 4, 3, 2)
    ).reshape(E, IT, 128, H).astype(bf16)
    w1slab = np.ascontiguousarray(
        W1d.reshape(E, IT, 128, HT, 128).transpose(0, 1, 4, 3, 2)
    ).reshape(E, IT, 128, H).astype(bf16)
    w2slab = np.ascontiguousarray(
        W2d.reshape(E, HT, 128, IT, 128).transpose(0, 1, 4, 3, 2)
    ).reshape(E, HT, 128, I).astype(bf16)

    xb = x.astype(bf16)

    in_maps = []
    for c in range(NCORES):
        pair = [expert_of[0][c], expert_of[1][c]]
        xt_c = np.zeros((2, H, CT), dtype=bf16)
        for s, e in enumerate(pair):
            n = len(cols[e])
            if n:
                xt_c[s, :, :n] = xb[cols[e]].T
        in_maps.append(
            {
                "xt": xt_c.reshape(2, HT, 128, CT),
                "w0t": w0slab[pair],
                "w1t": w1slab[pair],
                "w2t": w2slab[pair],
            }
        )

    nc = _get_built(jobs, CT)
    from concourse.bass_utils import run_bass_kernel_spmd

    res = run_bass_kernel_spmd(nc, in_maps, list(range(NCORES)))
    LAST_RESULTS = res

    # Y[e] = [H, CT] for expert e
    Y = np.empty((E, H, CT), dtype=np.float32)
    for c in range(NCORES):
        yt_c = np.asarray(res.results[c]["yt"]).reshape(2, H, CT)
        Y[expert_of[0][c]] = yt_c[0]
        Y[expert_of[1][c]] = yt_c[1]

    # ---- scatter back to [T, K, H] ----
    e_flat = sel.reshape(-1).astype(np.int64)
    t_flat = np.repeat(np.arange(T, dtype=np.int64), TOPK)
    p_flat = pos[e_flat, t_flat]
    out = Y[e_flat, :, p_flat]  # [T*K, H]
    return np.ascontiguousarray(out.reshape(T, TOPK, H), dtype=np.float32)


# revision 10
# speedup vs baseline: 1.0311x; 1.0311x over previous
"""DeepseekMoE block-quantized MoE kernel for 8 Trainium2 NeuronCores.

Strategy (expert-parallel with host-side dispatch):
  - The routing table (selected_experts) is known on the host before launch,
    so the all-to-all "dispatch" is done on the host: for each expert e we
    gather the unique tokens routed to it (dedup across the top-k slots),
    transpose to [H, n_e], and pad to a common capacity C.
  - Experts are sharded 2-per-core across the 8 cores.  Each core runs a
    dense 3-matmul MLP (gate/up -> silu*up -> down) for its 2 experts in
    x^T / act^T layout so no on-device transposes are needed.
  - Block-dequantization (w * repeat(s, 128)) is folded into the host-side
    weight preparation, which also rounds weights and x to bf16.
  - bf16 matmuls stream 1 column/cycle and enable Fast Weight Load
    (LDWEIGHTS ~53ns vs ~187ns for fp32r), so the stationary reload fully
    hides behind the moving-operand stream.  Accuracy budget: ~5e-3 L2.
  - Weights live in DRAM pre-swizzled into the exact SBUF slab layout so
    each slab load is one contiguous-per-partition DMA (4KB descriptors).
  - The host scatters the per-expert outputs back to [T, K, H].
"""

import math

import numpy as np

T = 4096
TOPK = 6
E = 16
H = 2048
I = 1408
BS = 128           # quant block size
HT = H // 128      # 16 h-tiles
IT = I // 128      # 11 i-tiles
NCORES = 8
# Single-pass SBUF budget bound: (HT + IT) * 2 * W bytes of x+act per
# partition plus ~50KB of weight/output staging must fit in ~208KB.
MAX_W = 2880

_BUILT = {}
LAST_RESULTS = None  # stashed BassKernelResults for external harnesses


def _chunk_plan(width):
    """Split `width` columns into PSUM-bank-sized chunks (<=512)."""
    if width <= 512:
        return [(0, width)]
    n = -(-width // 512)
    # 8-aligned chunk widths
    base = (width // n) // 8 * 8
    rem8 = (width - n * base) // 8
    out, off = [], 0
    for j in range(n):
        w = base + (8 if j < rem8 else 0)
        if j == n - 1:
            w = width - off
        out.append((off, w))
        off += w
    return out


def _build(jobs, CT):
    """Build the SPMD Bass program.  `jobs` is a tuple of
    (slot, col_offset, width): each job runs one expert slot's MLP over a
    window of `width` token columns; CT is the column capacity of xt/yt."""
    import concourse.bacc as bacc
    import concourse.mybir as mybir
    from concourse.bass import ts
    from concourse.tile import TileContext

    f32 = mybir.dt.float32
    bf16 = mybir.dt.bfloat16
    AF = mybir.ActivationFunctionType
    import os as _os

    act_fn = (
        AF.Sigmoid if _os.environ.get("KERNEL_SIM_SIGMOID") else AF.Silu
    )  # CoreSim lacks Silu; HW path always uses Silu

    nc = bacc.Bacc()
    xt = nc.declare_dram_parameter("xt", [2, HT, 128, CT], bf16, isOutput=False)
    # slab layouts: w0t[s, i, p, h*128+j] = W0deq[i*128+j, h*128+p]
    #               w2t[s, h, p, i*128+j] = W2deq[h*128+j, i*128+p]
    w0t = nc.declare_dram_parameter("w0t", [2, IT, 128, H], bf16, isOutput=False)
    w1t = nc.declare_dram_parameter("w1t", [2, IT, 128, H], bf16, isOutput=False)
    w2t = nc.declare_dram_parameter("w2t", [2, HT, 128, I], bf16, isOutput=False)
    yt = nc.declare_dram_parameter("yt", [2, HT, 128, CT], f32, isOutput=True)

    with TileContext(nc) as tc:
        with (
            tc.tile_pool(name="xp", bufs=1) as xp,
            tc.tile_pool(name="ap", bufs=1) as apool,
            tc.tile_pool(name="wp", bufs=2) as wp,
            tc.tile_pool(name="yp", bufs=2) as yp,
            tc.tile_pool(name="ps", bufs=2, space="PSUM") as ps,
        ):
            def load_w01_slab(which, src, sl, i, eng=None):
                slab = wp.tile([128, H], bf16, tag=which, name=None)
                (eng or nc.sync).dma_start(out=slab, in_=src[sl, i])
                return slab

            def new_xtile(jn, W):
                xs_all = xp.tile([128, HT * W], bf16, tag="x", name=f"x_{jn}")
                return xs_all, xs_all.rearrange("p (h w) -> p h w", h=HT)

            def xsrc(sl, co, c0, cw):
                return xt[sl, :, :, co + c0 : co + c0 + cw].rearrange(
                    "h p w -> p h w"
                )

            # DMA issue discipline (hard-won): transfers execute in strict
            # FIFO order per issuing queue, and the queues arbitrate fairly
            # -- so bulk prefetch on a parallel queue STARVES a just-in-time
            # stream.  Everything latency-critical therefore goes on the
            # sync queue in exact need-order; the scalar queue carries the
            # secondary stream (w0 tail, x c0 upper half, w1, x c2, y
            # stores) whose deadlines are looser.
            pending = None  # next job's preloaded tile handles
            for jn, (s, co, W) in enumerate(jobs):
                chunks = _chunk_plan(W)
                nch = len(chunks)
                if jn == 0:
                    xs_all, xs3 = new_xtile(0, W)
                    # startup split, need-ordered across both queues:
                    #   sync:   x c0 h0-3, x c0 h4-7, x c1
                    #   scalar: w0[0:128], w0[128:], x c0 h8-11, h12-15, w1, x c2
                    w0s_first = wp.tile([128, H], bf16, tag="w0")
                    nc.scalar.dma_start(
                        out=w0s_first[:, 0:128], in_=w0t[s, 0, :, 0:128]
                    )
                    c0, cw = chunks[0]
                    nc.sync.dma_start(
                        out=xs3[:, 0:4, c0 : c0 + cw],
                        in_=xsrc(s, co, c0, cw)[:, 0:4],
                    )
                    nc.scalar.dma_start(
                        out=w0s_first[:, 128:H], in_=w0t[s, 0, :, 128:H]
                    )
                    nc.sync.dma_start(
                        out=xs3[:, 4:8, c0 : c0 + cw],
                        in_=xsrc(s, co, c0, cw)[:, 4:8],
                    )
                    nc.scalar.dma_start(
                        out=xs3[:, 8:12, c0 : c0 + cw],
                        in_=xsrc(s, co, c0, cw)[:, 8:12],
                    )
                    if nch > 1:
                        c1, cw1 = chunks[1]
                        nc.sync.dma_start(
                            out=xs3[:, :, c1 : c1 + cw1], in_=xsrc(s, co, c1, cw1)
                        )
                    nc.scalar.dma_start(
                        out=xs3[:, 12:16, c0 : c0 + cw],
                        in_=xsrc(s, co, c0, cw)[:, 12:16],
                    )
                    w1s_first = load_w01_slab("w1", w1t, s, 0, eng=nc.scalar)
                    for c2, cw2 in chunks[2:]:
                        nc.scalar.dma_start(
                            out=xs3[:, :, c2 : c2 + cw2], in_=xsrc(s, co, c2, cw2)
                        )
                else:
                    xs_all, xs3, w0s_first, w1s_first = pending

                def xsl(h, c0, cw, xs_all=xs_all, W=W):
                    return xs_all[:, h * W + c0 : h * W + c0 + cw]

                acts_all = apool.tile([128, IT * W], bf16, tag="a", name=f"a_{jn}")

                def asl(i, c0, cw, acts_all=acts_all, W=W):
                    return acts_all[:, i * W + c0 : i * W + c0 + cw]

                # Phase A: gate/up projections + silu*up, per i-tile.
                for i in range(IT):
                    if i == 0:
                        w0s, w1s = w0s_first, w1s_first
                    else:
                        w0s = load_w01_slab("w0", w0t, s, i)
                        w1s = load_w01_slab("w1", w1t, s, i)
                    for c0, cw in chunks:
                        g = ps.tile([128, 512], f32, tag="g")
                        for h in range(HT):
                            nc.tensor.matmul(
                                g[:, :cw],
                                w0s[:, ts(h, 128)],
                                xsl(h, c0, cw),
                                start=(h == 0),
                                stop=(h == HT - 1),
                            )
                        u = ps.tile([128, 512], f32, tag="u")
                        for h in range(HT):
                            nc.tensor.matmul(
                                u[:, :cw],
                                w1s[:, ts(h, 128)],
                                xsl(h, c0, cw),
                                start=(h == 0),
                                stop=(h == HT - 1),
                            )
                        a_sl = asl(i, c0, cw)
                        nc.scalar.activation(a_sl, g[:, :cw], act_fn)
                        nc.vector.tensor_mul(a_sl, a_sl, u[:, :cw])

                # Next job's bulk loads, interleaved into this job's
                # phase-B emission on the sync queue so the w2 slabs ahead
                # of them always transfer first (see discipline note).
                emitters = {}
                if jn + 1 < len(jobs):
                    s2, co2, W2 = jobs[jn + 1]
                    nxt_x, nxt_x3 = new_xtile(jn + 1, W2)
                    ch2 = _chunk_plan(W2)
                    hpos = 1
                    for ci, (c0n, cwn) in enumerate(ch2):
                        def xe(c0n=c0n, cwn=cwn, s2=s2, co2=co2, x3=nxt_x3):
                            nc.sync.dma_start(
                                out=x3[:, :, c0n : c0n + cwn],
                                in_=xsrc(s2, co2, c0n, cwn),
                            )
                        emitters[hpos] = xe
                        hpos += 2
                    st = {}
                    def we0(st=st, s2=s2):
                        st["w0"] = load_w01_slab("w0", w0t, s2, 0)
                    def we1(st=st, s2=s2):
                        st["w1"] = load_w01_slab("w1", w1t, s2, 0)
                    emitters[hpos] = we0
                    emitters[hpos + 1] = we1

                # Phase B: down projection, per h-tile.  w2 slabs ride the
                # sync queue (deep prefetch, bufs=6); y stores issue from
                # the scalar queue, paced behind their PSUM->SBUF copies.
                for h in range(HT):
                    w2s = wp.tile([128, I], bf16, tag="w2", bufs=6)
                    nc.sync.dma_start(out=w2s, in_=w2t[s, h])
                    if h in emitters:
                        emitters[h]()
                    for c0, cw in chunks:
                        o = ps.tile([128, 512], f32, tag="o")
                        for i in range(IT):
                            nc.tensor.matmul(
                                o[:, :cw],
                                w2s[:, ts(i, 128)],
                                asl(i, c0, cw),
                                start=(i == 0),
                                stop=(i == IT - 1),
                            )
                        yc = yp.tile([128, 512], f32, tag="y")
                        nc.vector.tensor_copy(yc[:, :cw], o[:, :cw])
                        nc.scalar.dma_start(
                            out=yt[s, h, :, co + c0 : co + c0 + cw],
                            in_=yc[:, :cw],
                        )
                if jn + 1 < len(jobs):
                    pending = (nxt_x, nxt_x3, st["w0"], st["w1"])
    nc.finalize()
    return nc


def _get_built(jobs, CT):
    key = (tuple(jobs), CT)
    if key not in _BUILT:
        _BUILT[key] = _build(tuple(jobs), CT)
    return _BUILT[key]


def _dequant(w, s):
    """w: [E, O, Iin], s: [E, O, Iin//128] -> dequantized [E, O, Iin]."""
    e, o, iin = w.shape
    return (w.reshape(e, o, iin // BS, BS) * s[..., None]).reshape(e, o, iin)


def kernel(**inputs):
    global LAST_RESULTS
    import ml_dtypes

    bf16 = ml_dtypes.bfloat16

    x = np.ascontiguousarray(np.asarray(inputs["x"], dtype=np.float32))
    sel = np.asarray(inputs["selected_experts"])
    w0 = np.asarray(inputs["w0"], dtype=np.float32)
    s0 = np.asarray(inputs["s0"], dtype=np.float32)
    w1 = np.asarray(inputs["w1"], dtype=np.float32)
    s1 = np.asarray(inputs["s1"], dtype=np.float32)
    w2 = np.asarray(inputs["w2"], dtype=np.float32)
    s2 = np.asarray(inputs["s2"], dtype=np.float32)

    t, k = sel.shape
    assert (t, k) == (T, TOPK) and x.shape == (T, H)

    # ---- host-side dispatch: unique tokens per expert ----
    pos = np.full((E, T), -1, dtype=np.int32)
    cols = []
    for e in range(E):
        toks = np.nonzero((sel == e).any(axis=1))[0]
        cols.append(toks)
        pos[e, toks] = np.arange(len(toks), dtype=np.int32)
    counts = np.array([len(c) for c in cols])

    # Assign experts to (core, slot): slot 0 holds the 8 largest experts,
    # slot 1 the 8 smallest, so each slot's padded width is only the max of
    # its own rank group.  expert_of[s][c] = expert on core c, slot s.
    order = np.argsort(-counts, kind="stable")
    expert_of = [list(order[:NCORES]), list(order[NCORES:])]

    def align8(v):
        return max(256, -(-v // 8) * 8)

    slot_w = [align8(int(counts[expert_of[s]].max())) for s in range(2)]

    if max(slot_w) <= MAX_W:
        jobs = tuple((s, 0, slot_w[s]) for s in range(2))
        CT = max(slot_w)
    else:
        # fallback: uniform width, multiple column windows per slot
        cmax = int(counts.max())
        passes = max(1, math.ceil(cmax / MAX_W))
        W = align8(math.ceil(cmax / passes))
        CT = W * passes
        jobs = tuple((s, cp * W, W) for s in range(2) for cp in range(passes))

    # ---- dequantize + swizzle weights into SBUF slab layout (host) ----
    # w0/w1 slabs: [E, IT, 128(p=h-inner), HT*128(j of i-tile ... )]
    #   slab[e, it, p, ht*128+jj] = W0deq[e, it*128+jj, ht*128+p]
    W0d = _dequant(w0, s0)  # [E, I, H]
    W1d = _dequant(w1, s1)  # [E, I, H]
    W2d = _dequant(w2, s2)  # [E, H, I]
    w0slab = np.ascontiguousarray(
        W0d.reshape(E, IT, 128, HT, 128).transpose(0, 1, 4, 3, 2)
    ).reshape(E, IT, 128, H).astype(bf16)
    w1slab = np.ascontiguousarray(
        W1d.reshape(E, IT, 128, HT, 128).transpose(0, 1, 4, 3, 2)
    ).reshape(E, IT, 128, H).astype(bf16)
    w2slab = np.ascontiguousarray(
        W2d.reshape(E, HT, 128, IT, 128).transpose(0, 1, 4, 3, 2)
    ).reshape(E, HT, 128, I).astype(bf16)

    xb = x.astype(bf16)

    in_maps = []
    for c in range(NCORES):
        pair = [expert_of[0][c], expert_of[1][c]]
        xt_c = np.zeros((2, H, CT), dtype=bf16)
        for s, e in enumerate(pair):
            n = len(cols[e])
            if n:
                xt_c[s, :, :n] = xb[cols[e]].T
        in_maps.append(
            {
                "xt": xt_c.reshape(2, HT, 128, CT),
                "w0t": w0slab[pair],
                "w1t": w1slab[pair],
                "w2t": w2slab[pair],
            }
        )

    nc = _get_built(jobs, CT)
    from concourse.bass_utils import run_bass_kernel_spmd

    res = run_bass_kernel_spmd(nc, in_maps, list(range(NCORES)))
    LAST_RESULTS = res

    # Y[e] = [H, CT] for expert e
    Y = np.empty((E, H, CT), dtype=np.float32)
    for c in range(NCORES):
        yt_c = np.asarray(res.results[c]["yt"]).reshape(2, H, CT)
        Y[expert_of[0][c]] = yt_c[0]
        Y[expert_of[1][c]] = yt_c[1]

    # ---- scatter back to [T, K, H] ----
    e_flat = sel.reshape(-1).astype(np.int64)
    t_flat = np.repeat(np.arange(T, dtype=np.int64), TOPK)
    p_flat = pos[e_flat, t_flat]
    out = Y[e_flat, :, p_flat]  # [T*K, H]
    return np.ascontiguousarray(out.reshape(T, TOPK, H), dtype=np.float32)
